# revision 35
# baseline (speedup 1.0000x reference)
"""LoCon1d (position-specific conv1d) Trainium2 kernel.

out[b,o,s] = sum_{c,k} xpad[b,c,s+k] * w[o,c,s,k] + bias[o,s]
shapes: x (16,64,1024) f32, w (64,64,1024,3) f32, bias (64,1024) f32.

Sharding: sequence-parallel over 8 cores, 128 positions each.

Per-core mapping (shifted-stationary, diagonal extraction):
  Positions split into half-blocks (j, 64+j), j in 0..63, packed
  block-diagonally into the 128-partition contraction dim:
  partitions 0:64 = Cin for position j, 64:128 = Cin for position
  64+j; batch columns 0:16 <-> j, 16:32 <-> 64+j (zeros baked in on
  host).

  Groups of 4 consecutive pairs a in 0..3 (j = 4G+a). Per group, 3
  accumulating matmuls (one per tap kk) share one PSUM [128,256]:
    lhsT (stationary) = xt[:, 4G+kk : 4G+kk+4, :]  -> [128, 4*32]
    rhs  (moving)     = wr[:, G, kk, :]            -> [128, 4*64]
  Slot [32a'+m, 64a+o] accumulates sum_kk x[4G+kk+a'] * w[pair a];
  the diagonal a'==a holds the true output. The two copies per group
  read ps[0:64, 0:128] and ps[64:128, 128:256], each a [64,128]
  2-pair diagonal block identical in layout to a per-window output.
  Bias is added during host assembly.

  The kernel is HBM-bound (~4.2 MiB/core at ~0.13-0.14 MB/us per DMA
  path). sync, scalar and gpsimd are three independent DMA paths, so
  bytes are balanced across them in consumption order: x in two
  overlapping slabs, weights in small interleaved chunks, out DMAs
  alternating sync/scalar at the tail.
"""

import numpy as np

import concourse.bass as bass
import concourse.mybir as mybir
import concourse.tile as tile
from concourse import bacc, bass_utils

N_CORES = 8
B, CIN, COUT, S, K = 16, 64, 64, 1024, 3
SC = S // N_CORES          # positions per core (128)
H = SC // 2                # half-block (64)
W = H // 2                 # windows per core (32)
NG = W // 2                # matmul groups (16), 2 windows each
TW = H + K - 1             # x window length per half-block (66)
XSPLIT = 42                # xtA covers t<42 (G<=9), xtB t>=40 (G>=10)
XB0 = 40
OGROUPS = [8, 8, 8, 4, 4]  # out DMA group sizes in windows
FP8_TAP = 0                # this tap's weights ride in fp8 (e4m3)

_DT = {"f32": mybir.dt.float32, "bf16": mybir.dt.bfloat16,
       "f16": mybir.dt.float16}

DTYPE = "f16"


def _np_dt(dt):
    if dt == "bf16":
        import ml_dtypes
        return ml_dtypes.bfloat16
    if dt == "f16":
        return np.float16
    return np.float32


def build_bass(dtype=DTYPE):
    dt = _DT[dtype]
    nc = bacc.Bacc("TRN2", target_bir_lowering=False, debug=False,
                   num_devices=N_CORES)
    dt8 = mybir.dt.float8e4
    xra = nc.dram_tensor("xra", [128, XSPLIT, 32], dt, kind="ExternalInput")
    xrb = nc.dram_tensor("xrb", [128, TW - XB0, 32], dt,
                         kind="ExternalInput")
    wr = nc.dram_tensor("wr", [128, NG, K - 1, 256], dt,
                        kind="ExternalInput")
    wr8 = nc.dram_tensor("wr8", [128, NG, 256], dt8, kind="ExternalInput")
    out = nc.dram_tensor("out", [64, W, 128], dt, kind="ExternalOutput")

    # weight ranges (start window, n windows, path): balanced across the
    # three DMA paths in consumption order; gpsimd gets mid-stream and
    # tail work (its path has ~2us extra latency).
    wchunks = [(0, 4, 0), (4, 4, 1), (8, 4, 2), (12, 4, 0),
               (16, 4, 1), (20, 4, 2), (24, 4, 0), (28, 4, 2)]

    import contextlib
    with tile.TileContext(nc) as tc:
        n_sizes = sorted({c[1] for c in wchunks})
        with (
            tc.tile_pool(name="xapool", bufs=1) as xapool,
            tc.tile_pool(name="xbpool", bufs=1) as xbpool,
            tc.tile_pool(name="opool", bufs=len(OGROUPS)) as opool,
            tc.tile_pool(name="psum", bufs=8, space="PSUM") as pspool,
            contextlib.ExitStack() as wstack,
        ):
            wpools = {
                n: wstack.enter_context(tc.tile_pool(
                    name=f"w{n}",
                    bufs=sum(1 for c in wchunks if c[1] == n)))
                for n in n_sizes}

            w_engs = [nc.scalar, nc.sync, nc.gpsimd]
            xta = xapool.tile([128, XSPLIT, 32], dt)
            xtb = xbpool.tile([128, TW - XB0, 32], dt)
            nc.sync.dma_start(out=xta[:, :, :], in_=xra.ap())
            nc.scalar.dma_start(out=xtb[:, :, :], in_=xrb.ap())

            w_tiles = [None] * W     # per window: (f16 tile, fp8 tile, idx)
            for w0, nw, ei in wchunks:
                g0, g1 = w0 // 2, (w0 + nw) // 2
                wt = wpools[nw].tile([128, nw // 2, K - 1, 256], dt,
                                     tag=f"wt{nw}")
                w8t = wpools[nw].tile([128, nw // 2, 256], dt8,
                                      tag=f"w8t{nw}")
                w_engs[ei].dma_start(out=wt[:, :, :, :],
                                     in_=wr.ap()[:, g0:g1, :, :])
                w_engs[ei].dma_start(out=w8t[:, :, :],
                                     in_=wr8.ap()[:, g0:g1, :])
                for i in range(nw):
                    w_tiles[w0 + i] = (wt, w8t, i // 2)

            w = 0
            for gi, gn in enumerate(OGROUPS):
                o_t = opool.tile([64, gn, 128], dt, tag="ot")
                og0 = w
                for wi in range(0, gn, 2):
                    G = w // 2
                    wt, w8t, li = w_tiles[w]
                    if G < 10:
                        lhs_t = xta
                        t0 = 4 * G
                    else:
                        lhs_t = xtb
                        t0 = 4 * G - XB0
                    ps = pspool.tile([128, 256], mybir.dt.float32, tag="ps")
                    for kk in range(K):
                        if kk == FP8_TAP:
                            rhs = w8t[:, li, :]
                        else:
                            rhs = wt[:, li, kk - (1 if kk > FP8_TAP else 0), :]
                        nc.tensor.matmul(
                            ps[:, :],
                            lhsT=lhs_t[:, t0 + kk:t0 + kk + 4, :],
                            rhs=rhs,
                            start=(kk == 0),
                            stop=(kk == K - 1),
                        )
                    # two diagonal-pair blocks -> two window slots
                    nc.vector.tensor_copy(out=o_t[:, wi, :],
                                          in_=ps[0:64, 0:128])
                    nc.vector.tensor_copy(out=o_t[:, wi + 1, :],
                                          in_=ps[64:128, 128:256])
                    w += 2
                eng = nc.sync if gi % 2 == 0 else nc.scalar
                eng.dma_start(out=out.ap()[:, og0:og0 + gn, :],
                              in_=o_t[:, :, :])
    nc.compile()
    return nc


def prep_inputs(input, weight, bias, dtype=DTYPE):
    """Host-side shard + relayout. Returns list of per-core input dicts."""
    npdt = _np_dt(dtype)
    xpad = np.pad(np.asarray(input, np.float32), ((0, 0), (0, 0), (1, 1)))
    w = np.asarray(weight, np.float32)
    in_maps = []
    for i in range(N_CORES):
        s0 = i * SC
        # x: [p, t, b_ext] block-diagonal
        xa = xpad[:, :, s0:s0 + TW]             # (B, CIN, TW)
        xb = xpad[:, :, s0 + H:s0 + H + TW]
        xrc = np.zeros((128, TW, 32), np.float32)
        xrc[:64, :, :16] = xa.transpose(1, 2, 0)
        xrc[64:, :, 16:] = xb.transpose(1, 2, 0)
        # w: [p, G, kk, (a, o)] ; pair j = 4G + a
        ws = w[:, :, s0:s0 + SC, :]             # (COUT, CIN, SC, K)
        wa = ws[:, :, :H, :].reshape(COUT, CIN, NG, 4, K)
        wb = ws[:, :, H:, :].reshape(COUT, CIN, NG, 4, K)
        wrc = np.empty((128, NG, K, 4, COUT), np.float32)
        wrc[:64] = wa.transpose(1, 2, 4, 3, 0)  # (c, G, kk, a, o)
        wrc[64:] = wb.transpose(1, 2, 4, 3, 0)
        wrc = wrc.reshape(128, NG, K, 256)
        taps16 = [kk for kk in range(K) if kk != FP8_TAP]
        import ml_dtypes
        in_maps.append({
            "xra": np.ascontiguousarray(xrc[:, :XSPLIT, :].astype(npdt)),
            "xrb": np.ascontiguousarray(xrc[:, XB0:, :].astype(npdt)),
            "wr": np.ascontiguousarray(
                wrc[:, :, taps16, :].astype(npdt)),
            "wr8": np.ascontiguousarray(
                wrc[:, :, FP8_TAP, :].astype(ml_dtypes.float8_e4m3fn)),
        })
    return in_maps


def assemble_output(results, bias):
    full = np.empty((B, COUT, S), np.float32)
    for i, r in enumerate(results):
        s0 = i * SC
        oc = np.asarray(r["out"], np.float32)    # (64, W, 128)
        for p in range(2):
            blk = oc[32 * p:32 * p + 32, :, 64 * p:64 * p + 64]  # (m, w, o)
            # half A: positions s0 + 2w + p ; half B: s0 + 64 + 2w + p
            full[:, :, s0 + p:s0 + H:2] = blk[:16].transpose(0, 2, 1)
            full[:, :, s0 + H + p:s0 + SC:2] = blk[16:].transpose(0, 2, 1)
    full += np.asarray(bias, np.float32)[None, :, :]
    return full


_CACHED = {}


def run(inputs, dtype=DTYPE, trace=False):
    if dtype not in _CACHED:
        _CACHED[dtype] = build_bass(dtype)
    nc = _CACHED[dtype]
    in_maps = prep_inputs(inputs["input"], inputs["weight"], inputs["bias"],
                          dtype)
    res = bass_utils.run_bass_kernel_spmd(
        nc, in_maps, core_ids=list(range(N_CORES)), trace=trace)
    return assemble_output(res.results, inputs["bias"]), res


def kernel(input, weight, bias):
    out, _ = run({"input": input, "weight": weight, "bias": bias},
                 trace=False)
    return out


# revision 39
# speedup vs baseline: 1.1876x; 1.1876x over previous
"""LoCon1d (position-specific conv1d) Trainium2 kernel.

out[b,o,s] = sum_{c,k} xpad[b,c,s+k] * w[o,c,s,k] + bias[o,s]
shapes: x (16,64,1024) f32, w (64,64,1024,3) f32, bias (64,1024) f32.

Sharding: sequence-parallel over 8 cores, 128 positions each.

Per-core mapping (shifted-stationary, diagonal extraction):
  Positions split into half-blocks (j, 64+j), j in 0..63, packed
  block-diagonally into the 128-partition contraction dim:
  partitions 0:64 = Cin for position j, 64:128 = Cin for position
  64+j; batch columns 0:16 <-> j, 16:32 <-> 64+j (zeros baked in on
  host).

  Groups of 4 consecutive pairs a in 0..3 (j = 4G+a). Per group, 3
  accumulating matmuls (one per tap kk) share one PSUM [128,256]:
    lhsT (stationary) = xt[:, 4G+kk : 4G+kk+4, :]  -> [128, 4*32]
    rhs  (moving)     = wr[:, G, kk, :]            -> [128, 4*64]
  Slot [32a'+m, 64a+o] accumulates sum_kk x[4G+kk+a'] * w[pair a];
  the diagonal a'==a holds the true output. The two copies per group
  read ps[0:64, 0:128] and ps[64:128, 128:256], each a [64,128]
  2-pair diagonal block identical in layout to a per-window output.
  Bias is added during host assembly.

  The kernel is HBM-bound (~4.2 MiB/core at ~0.13-0.14 MB/us per DMA
  path). sync, scalar and gpsimd are three independent DMA paths, so
  bytes are balanced across them in consumption order: x in two
  overlapping slabs, weights in small interleaved chunks, out DMAs
  alternating sync/scalar at the tail.
"""

import numpy as np

import concourse.bass as bass
import concourse.mybir as mybir
import concourse.tile as tile
from concourse import bacc, bass_utils

N_CORES = 8
B, CIN, COUT, S, K = 16, 64, 64, 1024, 3
SC = S // N_CORES          # positions per core (128)
H = SC // 2                # half-block (64)
W = H // 2                 # windows per core (32)
NG = W // 2                # matmul groups (16), 2 windows each
TW = H + K - 1             # x window length per half-block (66)
XSPLIT = 42                # xtA covers t<42 (G<=9), xtB t>=40 (G>=10)
XB0 = 40
OGROUPS = [8, 8, 8, 4, 4]  # out DMA group sizes in windows
FP8_TAP = 0                # this tap's weights ride in fp8 (e4m3)

_DT = {"f32": mybir.dt.float32, "bf16": mybir.dt.bfloat16,
       "f16": mybir.dt.float16}

DTYPE = "f16"


def _np_dt(dt):
    if dt == "bf16":
        import ml_dtypes
        return ml_dtypes.bfloat16
    if dt == "f16":
        return np.float16
    return np.float32


def build_bass(dtype=DTYPE):
    dt = _DT[dtype]
    nc = bacc.Bacc("TRN2", target_bir_lowering=False, debug=False,
                   num_devices=N_CORES)
    dt8 = mybir.dt.float8e4
    xra = nc.dram_tensor("xra", [128, XSPLIT, 32], dt, kind="ExternalInput")
    xrb = nc.dram_tensor("xrb", [128, TW - XB0, 32], dt,
                         kind="ExternalInput")
    # packed per group: 512 f16 (taps != FP8_TAP) + 256 fp8 bytes viewed
    # as 128 f16 (tap FP8_TAP) -> 640 f16 per (partition, G)
    wr = nc.dram_tensor("wr", [128, NG, 640], dt, kind="ExternalInput")
    out = nc.dram_tensor("out", [64, W, 128], dt, kind="ExternalOutput")

    # weight ranges (start window, n windows, path): balanced across the
    # three DMA paths in consumption order; gpsimd gets mid-stream work
    # (its path has ~2us extra latency), <=5 DMAs outstanding per engine.
    wchunks = [(0, 4, 0), (4, 4, 1), (8, 4, 2), (12, 4, 0),
               (16, 4, 2), (20, 4, 2), (24, 4, 0), (28, 4, 1)]

    import contextlib
    with tile.TileContext(nc) as tc:
        n_sizes = sorted({c[1] for c in wchunks})
        with (
            tc.tile_pool(name="xapool", bufs=1) as xapool,
            tc.tile_pool(name="xbpool", bufs=1) as xbpool,
            tc.tile_pool(name="opool", bufs=len(OGROUPS)) as opool,
            tc.tile_pool(name="psum", bufs=8, space="PSUM") as pspool,
            contextlib.ExitStack() as wstack,
        ):
            wpools = {
                n: wstack.enter_context(tc.tile_pool(
                    name=f"w{n}",
                    bufs=sum(1 for c in wchunks if c[1] == n)))
                for n in n_sizes}

            w_engs = [nc.scalar, nc.sync, nc.gpsimd]
            xta = xapool.tile([128, XSPLIT, 32], dt)
            xtb = xbpool.tile([128, TW - XB0, 32], dt)
            nc.sync.dma_start(out=xta[:, :, :], in_=xra.ap())
            nc.scalar.dma_start(out=xtb[:, :, :], in_=xrb.ap())

            w_tiles = [None] * W     # per window: (packed tile, local idx)
            for w0, nw, ei in wchunks:
                g0, g1 = w0 // 2, (w0 + nw) // 2
                wt = wpools[nw].tile([128, nw // 2, 640], dt,
                                     tag=f"wt{nw}")
                w_engs[ei].dma_start(out=wt[:, :, :],
                                     in_=wr.ap()[:, g0:g1, :])
                for i in range(nw):
                    w_tiles[w0 + i] = (wt, i // 2)

            w = 0
            for gi, gn in enumerate(OGROUPS):
                o_t = opool.tile([64, gn, 128], dt, tag="ot")
                og0 = w
                for wi in range(0, gn, 2):
                    G = w // 2
                    wt, li = w_tiles[w]
                    if G < 10:
                        lhs_t = xta
                        t0 = 4 * G
                    else:
                        lhs_t = xtb
                        t0 = 4 * G - XB0
                    ps = pspool.tile([128, 256], mybir.dt.float32, tag="ps")
                    for kk in range(K):
                        if kk == FP8_TAP:
                            rhs = wt[:, li, 512:640].bitcast(dt8)
                        else:
                            ki = kk - (1 if kk > FP8_TAP else 0)
                            rhs = wt[:, li, 256 * ki:256 * ki + 256]
                        nc.tensor.matmul(
                            ps[:, :],
                            lhsT=lhs_t[:, t0 + kk:t0 + kk + 4, :],
                            rhs=rhs,
                            start=(kk == 0),
                            stop=(kk == K - 1),
                        )
                    # two diagonal-pair blocks -> two window slots; split
                    # the copies across vector and scalar
                    nc.vector.tensor_copy(out=o_t[:, wi, :],
                                          in_=ps[0:64, 0:128])
                    nc.scalar.copy(out=o_t[:, wi + 1, :],
                                   in_=ps[64:128, 128:256])
                    w += 2
                eng = nc.sync if gi % 2 == 0 else nc.scalar
                eng.dma_start(out=out.ap()[:, og0:og0 + gn, :],
                              in_=o_t[:, :, :])
    nc.compile()
    return nc


def prep_inputs(input, weight, bias, dtype=DTYPE):
    """Host-side shard + relayout. Returns list of per-core input dicts."""
    npdt = _np_dt(dtype)
    xpad = np.pad(np.asarray(input, np.float32), ((0, 0), (0, 0), (1, 1)))
    w = np.asarray(weight, np.float32)
    in_maps = []
    for i in range(N_CORES):
        s0 = i * SC
        # x: [p, t, b_ext] block-diagonal
        xa = xpad[:, :, s0:s0 + TW]             # (B, CIN, TW)
        xb = xpad[:, :, s0 + H:s0 + H + TW]
        xrc = np.zeros((128, TW, 32), np.float32)
        xrc[:64, :, :16] = xa.transpose(1, 2, 0)
        xrc[64:, :, 16:] = xb.transpose(1, 2, 0)
        # w: [p, G, kk, (a, o)] ; pair j = 4G + a
        ws = w[:, :, s0:s0 + SC, :]             # (COUT, CIN, SC, K)
        wa = ws[:, :, :H, :].reshape(COUT, CIN, NG, 4, K)
        wb = ws[:, :, H:, :].reshape(COUT, CIN, NG, 4, K)
        wrc = np.empty((128, NG, K, 4, COUT), np.float32)
        wrc[:64] = wa.transpose(1, 2, 4, 3, 0)  # (c, G, kk, a, o)
        wrc[64:] = wb.transpose(1, 2, 4, 3, 0)
        wrc = wrc.reshape(128, NG, K, 256)
        taps16 = [kk for kk in range(K) if kk != FP8_TAP]
        import ml_dtypes
        blob = np.empty((128, NG, 640), np.float16)
        blob[:, :, :512] = wrc[:, :, taps16, :].reshape(
            128, NG, 512).astype(np.float16)
        fp8b = np.ascontiguousarray(
            wrc[:, :, FP8_TAP, :].astype(ml_dtypes.float8_e4m3fn))
        blob[:, :, 512:] = fp8b.view(np.uint8).reshape(
            128, NG, 256).view(np.float16)
        in_maps.append({
            "xra": np.ascontiguousarray(xrc[:, :XSPLIT, :].astype(npdt)),
            "xrb": np.ascontiguousarray(xrc[:, XB0:, :].astype(npdt)),
            "wr": np.ascontiguousarray(blob),
        })
    return in_maps


def assemble_output(results, bias):
    full = np.empty((B, COUT, S), np.float32)
    for i, r in enumerate(results):
        s0 = i * SC
        oc = np.asarray(r["out"], np.float32)    # (64, W, 128)
        for p in range(2):
            blk = oc[32 * p:32 * p + 32, :, 64 * p:64 * p + 64]  # (m, w, o)
            # half A: positions s0 + 2w + p ; half B: s0 + 64 + 2w + p
            full[:, :, s0 + p:s0 + H:2] = blk[:16].transpose(0, 2, 1)
            full[:, :, s0 + H + p:s0 + SC:2] = blk[16:].transpose(0, 2, 1)
    full += np.asarray(bias, np.float32)[None, :, :]
    return full


_CACHED = {}


def run(inputs, dtype=DTYPE, trace=False):
    if dtype not in _CACHED:
        _CACHED[dtype] = build_bass(dtype)
    nc = _CACHED[dtype]
    in_maps = prep_inputs(inputs["input"], inputs["weight"], inputs["bias"],
                          dtype)
    res = bass_utils.run_bass_kernel_spmd(
        nc, in_maps, core_ids=list(range(N_CORES)), trace=trace)
    return assemble_output(res.results, inputs["bias"]), res


def kernel(input, weight, bias):
    out, _ = run({"input": input, "weight": weight, "bias": bias},
                 trace=False)
    return out
